# revision 8
# baseline (speedup 1.0000x reference)
"""Trainium2 Bass kernel for DynConv2d (DGCNN-style edge conv).

Reference computation (per batch b of 4):
  feats  = x[b,:,:,0].T                      # [N=8192, C=64]
  nn_idx = top16_j( 2*<f_i,f_j> - |f_i|^2 - |f_j|^2 )    # kNN graph
  edge   = [x_i, x_j - x_i] @ W.T + bias     # 1x1 conv, W [128, 128]
  out    = max over 16 neighbors             # -> [128, N]

Key algebraic reduction: with W = [W1 | W2],
  out[n, c] = u_n[c] + max_{j in top16(n)} v_j[c]
  u = (W1 - W2) @ feats.T + bias             # [128, N]
  v = W2 @ feats.T                           # [128, N]
so only a per-row top-16 over key[i, j] = <f_i, f_j> - 0.5*|f_j|^2 and a
gather+max over v remain.

Measured-HW design notes (per 128-row tile, per core):
 * Key matmuls: 3 accumulated bf16 matmuls on split operands
   (xh*yh + xh*yl + xl*yh; x = xh + xl + O(2^-17 x)) — bf16 matmul is
   ~0.5us vs 6.5us for fp32 on this part. Key error ~2^-17, far below
   typical top-16 decision gaps.
 * Top-16 selection (DVE): hierarchical. 8 group-wise max8 over 1024-wide
   groups (13.8us), then a 64-wide stage B: max8 -> top8 values, suppress
   them with is_ge mask + mult/add (match_replace has a ~5us fixed cost,
   the 2-op suppress is ~1us), max8 -> ranks 9-16. Index recovery with two
   full-width max_index scans (10.5us each). Replaces the flat 5-pass
   top-16 (50.5us) with ~38us.
 * v and u matmuls run in plain bf16 (error 2^-8 only lands on output
   values, tolerance is 2e-2).
 * Gather stays on gpsimd ap_gather (55us, runs parallel to DVE).

Sharding: 8 cores = 4 batches x 2 halves of N. Each core gets the full
feature matrix of its batch plus its local half of rows and produces
out[128, 4096]; the host concatenates. No collectives.
"""

import sys

for _p in ("/opt/trn_rl_repo", "/root/.axon_site/_ro/trn_rl_repo"):
    if _p not in sys.path:
        sys.path.insert(0, _p)

import numpy as np

B = 4
CIN = 64
COUT = 128
N = 8192
K = 16
N_CORES = 8

_prog_cache = {}


def build_program(n=N, r=N // 2, num_devices=N_CORES, repeat=1,
                  no_topk=False, no_gather=False, minimal=False,
                  tree_reduce=True):
    """Build + compile the SPMD bass program (same NEFF on all cores).

    repeat>1 wraps the main loop in a device-side For_i for benchmarking.
    no_topk/no_gather/minimal are benchmarking ablations (wrong results).
    tree_reduce: use 4 strided tensor_tensor max levels instead of
    reduce_max for the 16-neighbor max.
    """
    import concourse.bacc as bacc
    import concourse.mybir as mybir
    import concourse.tile as tile

    f32 = mybir.dt.float32
    bf16 = mybir.dt.bfloat16
    i16 = mybir.dt.int16
    u32 = mybir.dt.uint32
    Alu = mybir.AluOpType
    CH = 512
    nch = n // CH
    rt_count = r // 128
    NG = 8                    # groups for hierarchical top-16
    GS = n // NG              # group size (1024)

    nc = bacc.Bacc("TRN2", target_bir_lowering=False, debug=False,
                   num_devices=num_devices)

    feats_d = nc.dram_tensor("feats", [CIN, n], f32, kind="ExternalInput")
    featsl_d = nc.dram_tensor("featsl", [CIN, r], f32, kind="ExternalInput")
    w2t_d = nc.dram_tensor("w2t", [CIN, COUT], f32, kind="ExternalInput")
    wdt_d = nc.dram_tensor("wdt", [CIN, COUT], f32, kind="ExternalInput")
    bias_d = nc.dram_tensor("bias", [COUT, 1], f32, kind="ExternalInput")
    ident_d = nc.dram_tensor("ident", [128, 128], f32, kind="ExternalInput")
    out_d = nc.dram_tensor("out", [COUT, r], f32, kind="ExternalOutput")

    with tile.TileContext(nc) as tc:
        with tc.tile_pool(name="const", bufs=1) as const, \
             tc.tile_pool(name="keys", bufs=2) as keysp, \
             tc.tile_pool(name="vg", bufs=2) as vgp, \
             tc.tile_pool(name="small", bufs=3) as small, \
             tc.tile_pool(name="med", bufs=2) as med, \
             tc.tile_pool(name="tree", bufs=2) as treep, \
             tc.tile_pool(name="psk", bufs=4, space="PSUM") as psk, \
             tc.tile_pool(name="psa", bufs=2, space="PSUM") as psa:

            # ---------------- prologue ----------------
            # fp32 staging lives in borrowed "keys" slots (prologue only);
            # persistent tensors in the const pool.
            feats_aug = keysp.tile([CIN + 1, n], f32, tag="keys")
            nc.sync.dma_start(feats_aug[0:CIN, :], feats_d.ap())

            w2t = const.tile([CIN, COUT], bf16)
            w2f = med.tile([CIN, COUT], f32, tag="wstage")
            nc.sync.dma_start(w2f[:, :], w2t_d.ap())
            nc.vector.tensor_copy(w2t[:, :], w2f[:, :])
            wdt = const.tile([CIN, COUT], bf16)
            wdf = med.tile([CIN, COUT], f32, tag="wstage")
            nc.sync.dma_start(wdf[:, :], wdt_d.ap())
            nc.vector.tensor_copy(wdt[:, :], wdf[:, :])
            bias = const.tile([COUT, 1], f32)
            nc.sync.dma_start(bias[:, :], bias_d.ap())
            ident = const.tile([128, 128], f32)
            nc.sync.dma_start(ident[:, :], ident_d.ap())
            ones64 = const.tile([CIN, 1], f32)
            nc.vector.memset(ones64[:, :], 1.0)

            vt = const.tile([COUT, n], f32)
            ut = const.tile([COUT, r], f32)

            # |f_j|^2 row: square, then fp32 ones-matmul partition sum
            featsq = keysp.tile([CIN + 1, n], f32, tag="keys")
            nc.scalar.activation(featsq[0:CIN, :], feats_aug[0:CIN, :],
                                 mybir.ActivationFunctionType.Square)
            for c in range(nch):
                sl = slice(c * CH, (c + 1) * CH)
                pxx = psa.tile([1, CH], f32, tag="psa")
                nc.tensor.matmul(pxx[:, :], ones64[:, :], featsq[0:CIN, sl],
                                 start=True, stop=True)
                xs = med.tile([1, CH], f32, tag="xs")
                nc.scalar.activation(xs[:, :], pxx[:, :],
                                     mybir.ActivationFunctionType.Copy, scale=-0.5)
                # DMA shifts partition base: row 64 of feats_aug = -0.5*xx
                nc.sync.dma_start(feats_aug[CIN:CIN + 1, sl], xs[:, :])

            # bf16 split operands for the key matmul: x = H + L + O(2^-17 x)
            augH = const.tile([CIN + 1, n], bf16)
            augL = const.tile([CIN + 1, n], bf16)
            nc.vector.tensor_copy(augH[:, :], feats_aug[:, :])
            nc.vector.tensor_sub(augL[:, :], feats_aug[:, :], augH[:, :])
            oneH = const.tile([CIN + 1, r], bf16)
            oneL = const.tile([CIN + 1, r], bf16)
            # local rows fp32 staging reuses featsq's slot
            featsl_f32 = featsq
            nc.sync.dma_start(featsl_f32[0:CIN, 0:r], featsl_d.ap())
            nc.scalar.copy(oneH[0:CIN, :], featsl_f32[0:CIN, 0:r])
            nc.vector.memset(oneH[CIN:CIN + 1, :], 1.0)
            nc.vector.memset(oneL[CIN:CIN + 1, :], 0.0)
            nc.vector.tensor_sub(oneL[0:CIN, :], featsl_f32[0:CIN, 0:r],
                                 oneH[0:CIN, :])

            # v = W2 @ feats.T  (plain bf16; feeds only output values)
            for c in range(nch):
                sl = slice(c * CH, (c + 1) * CH)
                pv = psa.tile([COUT, CH], f32, tag="psa")
                nc.tensor.matmul(pv[:, :], w2t[:, :], augH[0:CIN, sl],
                                 start=True, stop=True)
                nc.scalar.copy(vt[:, sl], pv[:, :])

            # u = (W1-W2) @ featsl.T + bias  (plain bf16)
            for c in range(r // CH):
                sl = slice(c * CH, (c + 1) * CH)
                pu = psa.tile([COUT, CH], f32, tag="psa")
                nc.tensor.matmul(pu[:, :], wdt[:, :], oneH[0:CIN, sl],
                                 start=True, stop=True)
                nc.vector.tensor_scalar_add(ut[:, sl], pu[:, :], bias[:, :])

            # ---------------- main loop over row tiles ----------------
            # Software pipeline: stage A (PE keys matmuls + Act copies) for
            # tile rt+1 is emitted BEFORE stage B (DVE top-16 + gather chain)
            # of tile rt, so the per-engine program order has no cross-tile
            # stall: the tiny PE transpose of B(rt) — which waits on DVE —
            # sits after A(rt+1)'s matmuls, and the Pool gather stream stays
            # saturated.
            def stage_a(rt):
                rsl = slice(rt * 128, (rt + 1) * 128)
                keys = keysp.tile([128, n], f32, tag="keys")
                for c in range(nch):
                    sl = slice(c * CH, (c + 1) * CH)
                    pk = psk.tile([128, CH], f32, tag="psk")
                    nc.tensor.matmul(pk[:, :], oneH[:, rsl], augH[:, sl],
                                     start=True, stop=False)
                    nc.tensor.matmul(pk[:, :], oneH[:, rsl], augL[:, sl],
                                     start=False, stop=False)
                    nc.tensor.matmul(pk[:, :], oneL[:, rsl], augH[:, sl],
                                     start=False, stop=True)
                    nc.scalar.copy(keys[:, sl], pk[:, :])
                return keys

            def stage_b(rt, keys):
                rsl = slice(rt * 128, (rt + 1) * 128)
                if minimal:
                    ot0 = med.tile([128, 128], f32, tag="ot")
                    nc.vector.tensor_add(ot0[:, :], keys[:, 0:128], ut[:, rsl])
                    nc.sync.dma_start(out_d.ap()[:, rsl], ot0[:, :])
                    return

                jf = small.tile([128, 16], f32, tag="jf")
                if no_topk:
                    nc.vector.memset(jf[:, :], 5.0)
                else:
                    # stage A: top-8 of each of the 8 groups of 1024
                    gmax = small.tile([128, 8 * NG], f32, tag="gmax")
                    for g in range(NG):
                        nc.vector.max(gmax[:, 8 * g:8 * (g + 1)],
                                      keys[:, GS * g:GS * (g + 1)])
                    # stage B (64-wide): r1 = global top-8 values; suppress
                    # them (>= t8 -> -3e38); r2 = ranks 9-16
                    r1 = small.tile([128, 8], f32, tag="r8")
                    nc.vector.max(r1[:, :], gmax[:, :])
                    sup = small.tile([128, 8 * NG], f32, tag="sup")
                    nc.vector.tensor_scalar(sup[:, :], gmax[:, :],
                                            r1[:, 7:8], None, Alu.is_ge)
                    nc.vector.scalar_tensor_tensor(sup[:, :], sup[:, :],
                                                   -3.0e38, gmax[:, :],
                                                   Alu.mult, Alu.add)
                    r2 = small.tile([128, 8], f32, tag="r8")
                    nc.vector.max(r2[:, :], sup[:, :])
                    # index recovery: two full-width scans
                    i1 = small.tile([128, 8], u32, tag="i8")
                    nc.vector.max_index(i1[:, :], r1[:, :], keys[:, :])
                    i2 = small.tile([128, 8], u32, tag="i8")
                    nc.vector.max_index(i2[:, :], r2[:, :], keys[:, :])
                    nc.scalar.copy(jf[:, 0:8], i1[:, :])
                    nc.scalar.copy(jf[:, 8:16], i2[:, :])

                # wrapped int16 index layout for ap_gather:
                # widx[16g + q, m] = j[m, q] for all 8 groups g
                tp = psa.tile([16, 128], f32, tag="tp")
                nc.tensor.transpose(tp[:, :], jf[:, :], ident[:, :])
                tpi = small.tile([16, 128], i16, tag="tpi")
                nc.scalar.copy(tpi[:, :], tp[:, :])
                widx = small.tile([128, 128], i16, tag="widx")
                for g in range(8):
                    nc.sync.dma_start(widx[16 * g:16 * (g + 1), :], tpi[:, :])

                mx = med.tile([128, 128], f32, tag="mx")
                if no_gather:
                    nc.vector.tensor_copy(mx[:, :], ut[:, rsl])
                else:
                    # gather v columns of the 2048 neighbors, grouped max
                    vg = vgp.tile([128, 2048], f32, tag="vg")
                    nc.gpsimd.ap_gather(vg[:, :], vt[:, :], widx[:, :],
                                        channels=128, num_elems=n, d=1,
                                        num_idxs=2048)
                    if tree_reduce:
                        v3 = vg[:, :].rearrange("p (g k) -> p g k", k=K)
                        t1 = treep.tile([128, 1024], f32, tag="t1")
                        t1v = t1[:, :].rearrange("p (g k) -> p g k", k=8)
                        nc.vector.tensor_tensor(t1v, v3[:, :, 0:8],
                                                v3[:, :, 8:16], Alu.max)
                        t2 = treep.tile([128, 512], f32, tag="t2")
                        t2v = t2[:, :].rearrange("p (g k) -> p g k", k=4)
                        nc.vector.tensor_tensor(t2v, t1v[:, :, 0:4],
                                                t1v[:, :, 4:8], Alu.max)
                        t3 = treep.tile([128, 256], f32, tag="t3")
                        t3v = t3[:, :].rearrange("p (g k) -> p g k", k=2)
                        nc.vector.tensor_tensor(t3v, t2v[:, :, 0:2],
                                                t2v[:, :, 2:4], Alu.max)
                        nc.vector.tensor_tensor(mx[:, :], t3v[:, :, 0],
                                                t3v[:, :, 1], Alu.max)
                    else:
                        nc.vector.reduce_max(mx[:, :],
                                             vg[:, :].rearrange(
                                                 "p (g k) -> p g k", k=K),
                                             axis=mybir.AxisListType.X)
                ot = med.tile([128, 128], f32, tag="ot")
                nc.vector.tensor_add(ot[:, :], mx[:, :], ut[:, rsl])
                nc.sync.dma_start(out_d.ap()[:, rsl], ot[:, :])

            def main_body():
                keys_q = [stage_a(0)]
                for rt in range(rt_count):
                    if rt + 1 < rt_count:
                        keys_q.append(stage_a(rt + 1))
                    stage_b(rt, keys_q.pop(0))

            if repeat > 1:
                with tc.For_i(0, repeat, 1):
                    main_body()
            else:
                main_body()

    nc.compile()
    return nc


def _get_program(n, r, num_devices):
    key = (n, r, num_devices)
    if key not in _prog_cache:
        _prog_cache[key] = build_program(n, r, num_devices)
    return _prog_cache[key]


def run_cores(feats_by_core, featsl_by_core, W, b, n, r, trace=False):
    """Run the SPMD program. feats_by_core[i]: [64, n]; featsl_by_core[i]: [64, r]."""
    from concourse.bass_utils import run_bass_kernel_spmd

    num = len(feats_by_core)
    W1 = W[:, :CIN]
    W2 = W[:, CIN:]
    w2t = np.ascontiguousarray(W2.T).astype(np.float32)
    wdt = np.ascontiguousarray((W1 - W2).T).astype(np.float32)
    bias = b.reshape(COUT, 1).astype(np.float32)
    ident = np.eye(128, dtype=np.float32)
    in_maps = []
    for i in range(num):
        in_maps.append({
            "feats": np.ascontiguousarray(feats_by_core[i], dtype=np.float32),
            "featsl": np.ascontiguousarray(featsl_by_core[i], dtype=np.float32),
            "w2t": w2t, "wdt": wdt, "bias": bias, "ident": ident,
        })
    nc = _get_program(n, r, num)
    res = run_bass_kernel_spmd(nc, in_maps, core_ids=list(range(num)), trace=trace)
    return [res.results[i]["out"] for i in range(num)], res


def kernel(x, W, b):
    """Full-input entry point: x [4, 64, 8192, 1] f32 -> [4, 128, 8192, 1] f32."""
    x = np.asarray(x, dtype=np.float32)
    W = np.asarray(W, dtype=np.float32)
    b = np.asarray(b, dtype=np.float32)
    xb = np.ascontiguousarray(x[:, :, :, 0])            # [4, 64, 8192]
    r = N // 2
    feats_by_core = []
    featsl_by_core = []
    for core in range(N_CORES):
        bi, half = core // 2, core % 2
        feats_by_core.append(xb[bi])
        featsl_by_core.append(xb[bi][:, half * r:(half + 1) * r])
    outs, _ = run_cores(feats_by_core, featsl_by_core, W, b, N, r)
    out = np.empty((B, COUT, N, 1), np.float32)
    for core in range(N_CORES):
        bi, half = core // 2, core % 2
        out[bi, :, half * r:(half + 1) * r, 0] = outs[core]
    return out


# revision 10
# speedup vs baseline: 1.0667x; 1.0667x over previous
"""Trainium2 Bass kernel for DynConv2d (DGCNN-style edge conv).

Reference computation (per batch b of 4):
  feats  = x[b,:,:,0].T                      # [N=8192, C=64]
  nn_idx = top16_j( 2*<f_i,f_j> - |f_i|^2 - |f_j|^2 )    # kNN graph
  edge   = [x_i, x_j - x_i] @ W.T + bias     # 1x1 conv, W [128, 128]
  out    = max over 16 neighbors             # -> [128, N]

Key algebraic reduction: with W = [W1 | W2],
  out[n, c] = u_n[c] + max_{j in top16(n)} v_j[c]
  u = (W1 - W2) @ feats.T + bias             # [128, N]
  v = W2 @ feats.T                           # [128, N]
so only a per-row top-16 over key[i, j] = <f_i, f_j> - 0.5*|f_j|^2 and a
gather+max over v remain.

Measured-HW design notes (per 128-row tile, per core):
 * Key matmuls: 3 accumulated bf16 matmuls on split operands
   (xh*yh + xh*yl + xl*yh; x = xh + xl + O(2^-17 x)) — bf16 matmul is
   ~0.5us vs 6.5us for fp32 on this part. Key error ~2^-17, far below
   typical top-16 decision gaps.
 * Top-16 selection (DVE): hierarchical. 8 group-wise max8 over 1024-wide
   groups (13.8us), then a 64-wide stage B: max8 -> top8 values, suppress
   them with is_ge mask + mult/add (match_replace has a ~5us fixed cost,
   the 2-op suppress is ~1us), max8 -> ranks 9-16. Index recovery with two
   full-width max_index scans (10.5us each). Replaces the flat 5-pass
   top-16 (50.5us) with ~38us.
 * v and u matmuls run in plain bf16 (error 2^-8 only lands on output
   values, tolerance is 2e-2).
 * Gather stays on gpsimd ap_gather (55us, runs parallel to DVE).

Sharding: 8 cores = 4 batches x 2 halves of N. Each core gets the full
feature matrix of its batch plus its local half of rows and produces
out[128, 4096]; the host concatenates. No collectives.
"""

import sys

for _p in ("/opt/trn_rl_repo", "/root/.axon_site/_ro/trn_rl_repo"):
    if _p not in sys.path:
        sys.path.insert(0, _p)

import numpy as np

B = 4
CIN = 64
COUT = 128
N = 8192
K = 16
N_CORES = 8

_prog_cache = {}


def build_program(n=N, r=N // 2, num_devices=N_CORES, repeat=1,
                  no_topk=False, no_gather=False, minimal=False,
                  tree_reduce=True):
    """Build + compile the SPMD bass program (same NEFF on all cores).

    repeat>1 wraps the main loop in a device-side For_i for benchmarking.
    no_topk/no_gather/minimal are benchmarking ablations (wrong results).
    tree_reduce: use 4 strided tensor_tensor max levels instead of
    reduce_max for the 16-neighbor max.
    """
    import concourse.bacc as bacc
    import concourse.mybir as mybir
    import concourse.tile as tile

    f32 = mybir.dt.float32
    bf16 = mybir.dt.bfloat16
    i16 = mybir.dt.int16
    u32 = mybir.dt.uint32
    Alu = mybir.AluOpType
    CH = 512
    nch = n // CH
    rt_count = r // 128
    NG = 8                    # groups for hierarchical top-16
    GS = n // NG              # group size (1024)

    nc = bacc.Bacc("TRN2", target_bir_lowering=False, debug=False,
                   num_devices=num_devices)

    feats_d = nc.dram_tensor("feats", [CIN, n], f32, kind="ExternalInput")
    featsl_d = nc.dram_tensor("featsl", [CIN, r], f32, kind="ExternalInput")
    w2t_d = nc.dram_tensor("w2t", [CIN, COUT], f32, kind="ExternalInput")
    wdt_d = nc.dram_tensor("wdt", [CIN, COUT], f32, kind="ExternalInput")
    bias_d = nc.dram_tensor("bias", [COUT, 1], f32, kind="ExternalInput")
    ident_d = nc.dram_tensor("ident", [128, 128], f32, kind="ExternalInput")
    out_d = nc.dram_tensor("out", [COUT, r], f32, kind="ExternalOutput")

    with tile.TileContext(nc) as tc:
        with tc.tile_pool(name="const", bufs=1) as const, \
             tc.tile_pool(name="keys", bufs=2) as keysp, \
             tc.tile_pool(name="vg", bufs=2) as vgp, \
             tc.tile_pool(name="small", bufs=3) as small, \
             tc.tile_pool(name="med", bufs=2) as med, \
             tc.tile_pool(name="tree", bufs=2) as treep, \
             tc.tile_pool(name="psk", bufs=4, space="PSUM") as psk, \
             tc.tile_pool(name="psa", bufs=2, space="PSUM") as psa:

            # ---------------- prologue ----------------
            # fp32 staging lives in borrowed "keys" slots (prologue only);
            # persistent tensors in the const pool.
            feats_aug = keysp.tile([CIN + 1, n], f32, tag="keys")
            nc.sync.dma_start(feats_aug[0:CIN, :], feats_d.ap())

            w2t = const.tile([CIN, COUT], bf16)
            w2f = med.tile([CIN, COUT], f32, tag="wstage")
            nc.sync.dma_start(w2f[:, :], w2t_d.ap())
            nc.vector.tensor_copy(w2t[:, :], w2f[:, :])
            wdt = const.tile([CIN, COUT], bf16)
            wdf = med.tile([CIN, COUT], f32, tag="wstage")
            nc.sync.dma_start(wdf[:, :], wdt_d.ap())
            nc.vector.tensor_copy(wdt[:, :], wdf[:, :])
            bias = const.tile([COUT, 1], f32)
            nc.sync.dma_start(bias[:, :], bias_d.ap())
            ident = const.tile([128, 128], f32)
            nc.sync.dma_start(ident[:, :], ident_d.ap())
            ones64 = const.tile([CIN, 1], f32)
            nc.vector.memset(ones64[:, :], 1.0)

            vt = const.tile([COUT, n], f32)
            ut = const.tile([COUT, r], f32)

            # |f_j|^2 row: square, then fp32 ones-matmul partition sum
            featsq = keysp.tile([CIN + 1, n], f32, tag="keys")
            nc.scalar.activation(featsq[0:CIN, :], feats_aug[0:CIN, :],
                                 mybir.ActivationFunctionType.Square)
            for c in range(nch):
                sl = slice(c * CH, (c + 1) * CH)
                pxx = psa.tile([1, CH], f32, tag="psa")
                nc.tensor.matmul(pxx[:, :], ones64[:, :], featsq[0:CIN, sl],
                                 start=True, stop=True)
                xs = med.tile([1, CH], f32, tag="xs")
                nc.scalar.activation(xs[:, :], pxx[:, :],
                                     mybir.ActivationFunctionType.Copy, scale=-0.5)
                # DMA shifts partition base: row 64 of feats_aug = -0.5*xx
                nc.sync.dma_start(feats_aug[CIN:CIN + 1, sl], xs[:, :])

            # bf16 split operands for the key matmul: x = H + L + O(2^-17 x)
            augH = const.tile([CIN + 1, n], bf16)
            augL = const.tile([CIN + 1, n], bf16)
            nc.vector.tensor_copy(augH[:, :], feats_aug[:, :])
            nc.vector.tensor_sub(augL[:, :], feats_aug[:, :], augH[:, :])
            oneH = const.tile([CIN + 1, r], bf16)
            oneL = const.tile([CIN + 1, r], bf16)
            # local rows fp32 staging reuses featsq's slot
            featsl_f32 = featsq
            nc.sync.dma_start(featsl_f32[0:CIN, 0:r], featsl_d.ap())
            nc.scalar.copy(oneH[0:CIN, :], featsl_f32[0:CIN, 0:r])
            nc.vector.memset(oneH[CIN:CIN + 1, :], 1.0)
            nc.vector.memset(oneL[CIN:CIN + 1, :], 0.0)
            nc.vector.tensor_sub(oneL[0:CIN, :], featsl_f32[0:CIN, 0:r],
                                 oneH[0:CIN, :])

            # v = W2 @ feats.T  (plain bf16; feeds only output values)
            for c in range(nch):
                sl = slice(c * CH, (c + 1) * CH)
                pv = psa.tile([COUT, CH], f32, tag="psa")
                nc.tensor.matmul(pv[:, :], w2t[:, :], augH[0:CIN, sl],
                                 start=True, stop=True)
                nc.scalar.copy(vt[:, sl], pv[:, :])

            # u = (W1-W2) @ featsl.T + bias  (plain bf16)
            for c in range(r // CH):
                sl = slice(c * CH, (c + 1) * CH)
                pu = psa.tile([COUT, CH], f32, tag="psa")
                nc.tensor.matmul(pu[:, :], wdt[:, :], oneH[0:CIN, sl],
                                 start=True, stop=True)
                nc.vector.tensor_scalar_add(ut[:, sl], pu[:, :], bias[:, :])

            # ---------------- main loop over row tiles ----------------
            # Software pipeline: stage A (PE keys matmuls + Act copies) for
            # tile rt+1 is emitted BEFORE stage B (DVE top-16 + gather chain)
            # of tile rt, so the per-engine program order has no cross-tile
            # stall: the tiny PE transpose of B(rt) — which waits on DVE —
            # sits after A(rt+1)'s matmuls, and the Pool gather stream stays
            # saturated.
            def stage_a(rt):
                rsl = slice(rt * 128, (rt + 1) * 128)
                keys = keysp.tile([128, n], f32, tag="keys")
                for c in range(nch):
                    sl = slice(c * CH, (c + 1) * CH)
                    pk = psk.tile([128, CH], f32, tag="psk")
                    nc.tensor.matmul(pk[:, :], oneH[:, rsl], augH[:, sl],
                                     start=True, stop=False)
                    nc.tensor.matmul(pk[:, :], oneH[:, rsl], augL[:, sl],
                                     start=False, stop=False)
                    nc.tensor.matmul(pk[:, :], oneL[:, rsl], augH[:, sl],
                                     start=False, stop=True)
                    nc.scalar.copy(keys[:, sl], pk[:, :])
                return keys

            def stage_b(rt, keys):
                rsl = slice(rt * 128, (rt + 1) * 128)
                if minimal:
                    ot0 = med.tile([128, 128], f32, tag="ot")
                    nc.vector.tensor_add(ot0[:, :], keys[:, 0:128], ut[:, rsl])
                    nc.sync.dma_start(out_d.ap()[:, rsl], ot0[:, :])
                    return

                # jfw holds the 16 per-row indices replicated 8x along the
                # free dim: jfw[m, 16g+q] = j[m, q]. One PE transpose then
                # yields the wrapped ap_gather index layout directly
                # (widx[16g+q, m] = j[m, q]) with no partition-shifting DMAs.
                jfw = small.tile([128, 128], f32, tag="jfw")
                if no_topk:
                    nc.vector.memset(jfw[:, :], 5.0)
                else:
                    # stage A: top-8 of each of the 8 groups of 1024
                    gmax = small.tile([128, 8 * NG], f32, tag="gmax")
                    for g in range(NG):
                        nc.vector.max(gmax[:, 8 * g:8 * (g + 1)],
                                      keys[:, GS * g:GS * (g + 1)])
                    # stage B (64-wide): r1 = global top-8 values; suppress
                    # them (>= t8 -> -3e38); r2 = ranks 9-16
                    r1 = small.tile([128, 8], f32, tag="r8")
                    nc.vector.max(r1[:, :], gmax[:, :])
                    sup = small.tile([128, 8 * NG], f32, tag="sup")
                    nc.vector.tensor_scalar(sup[:, :], gmax[:, :],
                                            r1[:, 7:8], None, Alu.is_ge)
                    nc.vector.scalar_tensor_tensor(sup[:, :], sup[:, :],
                                                   -3.0e38, gmax[:, :],
                                                   Alu.mult, Alu.add)
                    r2 = small.tile([128, 8], f32, tag="r8")
                    nc.vector.max(r2[:, :], sup[:, :])
                    # index recovery: two full-width scans
                    i1 = small.tile([128, 8], u32, tag="i8")
                    nc.vector.max_index(i1[:, :], r1[:, :], keys[:, :])
                    i2 = small.tile([128, 8], u32, tag="i8")
                    nc.vector.max_index(i2[:, :], r2[:, :], keys[:, :])
                    for g in range(8):
                        nc.scalar.copy(jfw[:, 16 * g:16 * g + 8], i1[:, :])
                        nc.scalar.copy(jfw[:, 16 * g + 8:16 * g + 16], i2[:, :])

                tp = psa.tile([128, 128], f32, tag="tp")
                nc.tensor.transpose(tp[:, :], jfw[:, :], ident[:, :])
                widx = small.tile([128, 128], i16, tag="widx")
                nc.scalar.copy(widx[:, :], tp[:, :])

                mx = med.tile([128, 128], f32, tag="mx")
                if no_gather:
                    nc.vector.tensor_copy(mx[:, :], ut[:, rsl])
                else:
                    # gather v columns of the 2048 neighbors, grouped max
                    vg = vgp.tile([128, 2048], f32, tag="vg")
                    nc.gpsimd.ap_gather(vg[:, :], vt[:, :], widx[:, :],
                                        channels=128, num_elems=n, d=1,
                                        num_idxs=2048)
                    if tree_reduce:
                        v3 = vg[:, :].rearrange("p (g k) -> p g k", k=K)
                        t1 = treep.tile([128, 1024], f32, tag="t1")
                        t1v = t1[:, :].rearrange("p (g k) -> p g k", k=8)
                        nc.vector.tensor_tensor(t1v, v3[:, :, 0:8],
                                                v3[:, :, 8:16], Alu.max)
                        t2 = treep.tile([128, 512], f32, tag="t2")
                        t2v = t2[:, :].rearrange("p (g k) -> p g k", k=4)
                        nc.vector.tensor_tensor(t2v, t1v[:, :, 0:4],
                                                t1v[:, :, 4:8], Alu.max)
                        t3 = treep.tile([128, 256], f32, tag="t3")
                        t3v = t3[:, :].rearrange("p (g k) -> p g k", k=2)
                        nc.vector.tensor_tensor(t3v, t2v[:, :, 0:2],
                                                t2v[:, :, 2:4], Alu.max)
                        nc.vector.tensor_tensor(mx[:, :], t3v[:, :, 0],
                                                t3v[:, :, 1], Alu.max)
                    else:
                        nc.vector.reduce_max(mx[:, :],
                                             vg[:, :].rearrange(
                                                 "p (g k) -> p g k", k=K),
                                             axis=mybir.AxisListType.X)
                ot = med.tile([128, 128], f32, tag="ot")
                nc.vector.tensor_add(ot[:, :], mx[:, :], ut[:, rsl])
                nc.sync.dma_start(out_d.ap()[:, rsl], ot[:, :])

            def main_body():
                keys_q = [stage_a(0)]
                for rt in range(rt_count):
                    if rt + 1 < rt_count:
                        keys_q.append(stage_a(rt + 1))
                    stage_b(rt, keys_q.pop(0))

            if repeat > 1:
                with tc.For_i(0, repeat, 1):
                    main_body()
            else:
                main_body()

    nc.compile()
    return nc


def _get_program(n, r, num_devices):
    key = (n, r, num_devices)
    if key not in _prog_cache:
        _prog_cache[key] = build_program(n, r, num_devices)
    return _prog_cache[key]


def run_cores(feats_by_core, featsl_by_core, W, b, n, r, trace=False):
    """Run the SPMD program. feats_by_core[i]: [64, n]; featsl_by_core[i]: [64, r]."""
    from concourse.bass_utils import run_bass_kernel_spmd

    num = len(feats_by_core)
    W1 = W[:, :CIN]
    W2 = W[:, CIN:]
    w2t = np.ascontiguousarray(W2.T).astype(np.float32)
    wdt = np.ascontiguousarray((W1 - W2).T).astype(np.float32)
    bias = b.reshape(COUT, 1).astype(np.float32)
    ident = np.eye(128, dtype=np.float32)
    in_maps = []
    for i in range(num):
        in_maps.append({
            "feats": np.ascontiguousarray(feats_by_core[i], dtype=np.float32),
            "featsl": np.ascontiguousarray(featsl_by_core[i], dtype=np.float32),
            "w2t": w2t, "wdt": wdt, "bias": bias, "ident": ident,
        })
    nc = _get_program(n, r, num)
    res = run_bass_kernel_spmd(nc, in_maps, core_ids=list(range(num)), trace=trace)
    return [res.results[i]["out"] for i in range(num)], res


def kernel(x, W, b):
    """Full-input entry point: x [4, 64, 8192, 1] f32 -> [4, 128, 8192, 1] f32."""
    x = np.asarray(x, dtype=np.float32)
    W = np.asarray(W, dtype=np.float32)
    b = np.asarray(b, dtype=np.float32)
    xb = np.ascontiguousarray(x[:, :, :, 0])            # [4, 64, 8192]
    r = N // 2
    feats_by_core = []
    featsl_by_core = []
    for core in range(N_CORES):
        bi, half = core // 2, core % 2
        feats_by_core.append(xb[bi])
        featsl_by_core.append(xb[bi][:, half * r:(half + 1) * r])
    outs, _ = run_cores(feats_by_core, featsl_by_core, W, b, N, r)
    out = np.empty((B, COUT, N, 1), np.float32)
    for core in range(N_CORES):
        bi, half = core // 2, core % 2
        out[bi, :, half * r:(half + 1) * r, 0] = outs[core]
    return out


# revision 11
# speedup vs baseline: 1.0825x; 1.0148x over previous
"""Trainium2 Bass kernel for DynConv2d (DGCNN-style edge conv).

Reference computation (per batch b of 4):
  feats  = x[b,:,:,0].T                      # [N=8192, C=64]
  nn_idx = top16_j( 2*<f_i,f_j> - |f_i|^2 - |f_j|^2 )    # kNN graph
  edge   = [x_i, x_j - x_i] @ W.T + bias     # 1x1 conv, W [128, 128]
  out    = max over 16 neighbors             # -> [128, N]

Key algebraic reduction: with W = [W1 | W2],
  out[n, c] = u_n[c] + max_{j in top16(n)} v_j[c]
  u = (W1 - W2) @ feats.T + bias             # [128, N]
  v = W2 @ feats.T                           # [128, N]
so only a per-row top-16 over key[i, j] = <f_i, f_j> - 0.5*|f_j|^2 and a
gather+max over v remain.

Measured-HW design notes (per 128-row tile, per core):
 * Key matmuls: 3 accumulated bf16 matmuls on split operands
   (xh*yh + xh*yl + xl*yh; x = xh + xl + O(2^-17 x)) — bf16 matmul is
   ~0.5us vs 6.5us for fp32 on this part. Key error ~2^-17, far below
   typical top-16 decision gaps.
 * Top-16 selection (DVE): hierarchical. 8 group-wise max8 over 1024-wide
   groups (13.8us), then a 64-wide stage B: max8 -> top8 values, suppress
   them with is_ge mask + mult/add (match_replace has a ~5us fixed cost,
   the 2-op suppress is ~1us), max8 -> ranks 9-16. Index recovery with two
   full-width max_index scans (10.5us each). Replaces the flat 5-pass
   top-16 (50.5us) with ~38us.
 * v and u matmuls run in plain bf16 (error 2^-8 only lands on output
   values, tolerance is 2e-2).
 * Gather stays on gpsimd ap_gather (55us, runs parallel to DVE).

Sharding: 8 cores = 4 batches x 2 halves of N. Each core gets the full
feature matrix of its batch plus its local half of rows and produces
out[128, 4096]; the host concatenates. No collectives.
"""

import sys

for _p in ("/opt/trn_rl_repo", "/root/.axon_site/_ro/trn_rl_repo"):
    if _p not in sys.path:
        sys.path.insert(0, _p)

import numpy as np

B = 4
CIN = 64
COUT = 128
N = 8192
K = 16
N_CORES = 8

_prog_cache = {}


def build_program(n=N, r=N // 2, num_devices=N_CORES, repeat=1,
                  no_topk=False, no_gather=False, minimal=False,
                  tree_reduce=True):
    """Build + compile the SPMD bass program (same NEFF on all cores).

    repeat>1 wraps the main loop in a device-side For_i for benchmarking.
    no_topk/no_gather/minimal are benchmarking ablations (wrong results).
    tree_reduce: use 4 strided tensor_tensor max levels instead of
    reduce_max for the 16-neighbor max.
    """
    import concourse.bacc as bacc
    import concourse.mybir as mybir
    import concourse.tile as tile

    f32 = mybir.dt.float32
    bf16 = mybir.dt.bfloat16
    i16 = mybir.dt.int16
    u32 = mybir.dt.uint32
    Alu = mybir.AluOpType
    CH = 512
    nch = n // CH
    rt_count = r // 128
    NG = 8                    # groups for hierarchical top-16
    GS = n // NG              # group size (1024)

    nc = bacc.Bacc("TRN2", target_bir_lowering=False, debug=False,
                   num_devices=num_devices)

    feats_d = nc.dram_tensor("feats", [CIN, n], f32, kind="ExternalInput")
    featsl_d = nc.dram_tensor("featsl", [CIN, r], f32, kind="ExternalInput")
    w2t_d = nc.dram_tensor("w2t", [CIN, COUT], f32, kind="ExternalInput")
    wdt_d = nc.dram_tensor("wdt", [CIN, COUT], f32, kind="ExternalInput")
    bias_d = nc.dram_tensor("bias", [COUT, 1], f32, kind="ExternalInput")
    ident_d = nc.dram_tensor("ident", [128, 128], f32, kind="ExternalInput")
    out_d = nc.dram_tensor("out", [COUT, r], f32, kind="ExternalOutput")

    with tile.TileContext(nc) as tc:
        with tc.tile_pool(name="const", bufs=1) as const, \
             tc.tile_pool(name="keys", bufs=2) as keysp, \
             tc.tile_pool(name="vg", bufs=2) as vgp, \
             tc.tile_pool(name="small", bufs=3) as small, \
             tc.tile_pool(name="med", bufs=2) as med, \
             tc.tile_pool(name="tree", bufs=2) as treep, \
             tc.tile_pool(name="psk", bufs=4, space="PSUM") as psk, \
             tc.tile_pool(name="psa", bufs=2, space="PSUM") as psa:

            # ---------------- prologue ----------------
            # fp32 staging lives in borrowed "keys" slots (prologue only);
            # persistent tensors in the const pool.
            feats_aug = keysp.tile([CIN + 1, n], f32, tag="keys")
            nc.sync.dma_start(feats_aug[0:CIN, :], feats_d.ap())

            w2t = const.tile([CIN, COUT], bf16)
            w2f = med.tile([CIN, COUT], f32, tag="wstage")
            nc.sync.dma_start(w2f[:, :], w2t_d.ap())
            nc.vector.tensor_copy(w2t[:, :], w2f[:, :])
            wdt = const.tile([CIN, COUT], bf16)
            wdf = med.tile([CIN, COUT], f32, tag="wstage")
            nc.sync.dma_start(wdf[:, :], wdt_d.ap())
            nc.vector.tensor_copy(wdt[:, :], wdf[:, :])
            bias = const.tile([COUT, 1], f32)
            nc.sync.dma_start(bias[:, :], bias_d.ap())
            ident = const.tile([128, 128], f32)
            nc.sync.dma_start(ident[:, :], ident_d.ap())
            ones64 = const.tile([CIN, 1], f32)
            nc.vector.memset(ones64[:, :], 1.0)

            vt = const.tile([COUT, n], f32)
            ut = const.tile([COUT, r], f32)

            # |f_j|^2 row: square, then fp32 ones-matmul partition sum
            featsq = keysp.tile([CIN + 1, n], f32, tag="keys")
            nc.scalar.activation(featsq[0:CIN, :], feats_aug[0:CIN, :],
                                 mybir.ActivationFunctionType.Square)
            for c in range(nch):
                sl = slice(c * CH, (c + 1) * CH)
                pxx = psa.tile([1, CH], f32, tag="psa")
                nc.tensor.matmul(pxx[:, :], ones64[:, :], featsq[0:CIN, sl],
                                 start=True, stop=True)
                xs = med.tile([1, CH], f32, tag="xs")
                nc.scalar.activation(xs[:, :], pxx[:, :],
                                     mybir.ActivationFunctionType.Copy, scale=-0.5)
                # DMA shifts partition base: row 64 of feats_aug = -0.5*xx
                nc.sync.dma_start(feats_aug[CIN:CIN + 1, sl], xs[:, :])

            # bf16 split operands for the key matmul: x = H + L + O(2^-17 x)
            augH = const.tile([CIN + 1, n], bf16)
            augL = const.tile([CIN + 1, n], bf16)
            nc.vector.tensor_copy(augH[:, :], feats_aug[:, :])
            nc.vector.tensor_sub(augL[:, :], feats_aug[:, :], augH[:, :])
            oneH = const.tile([CIN + 1, r], bf16)
            oneL = const.tile([CIN + 1, r], bf16)
            # local rows fp32 staging reuses featsq's slot
            featsl_f32 = featsq
            nc.sync.dma_start(featsl_f32[0:CIN, 0:r], featsl_d.ap())
            nc.scalar.copy(oneH[0:CIN, :], featsl_f32[0:CIN, 0:r])
            nc.vector.memset(oneH[CIN:CIN + 1, :], 1.0)
            nc.vector.memset(oneL[CIN:CIN + 1, :], 0.0)
            nc.vector.tensor_sub(oneL[0:CIN, :], featsl_f32[0:CIN, 0:r],
                                 oneH[0:CIN, :])

            # v = W2 @ feats.T  (plain bf16; feeds only output values)
            for c in range(nch):
                sl = slice(c * CH, (c + 1) * CH)
                pv = psa.tile([COUT, CH], f32, tag="psa")
                nc.tensor.matmul(pv[:, :], w2t[:, :], augH[0:CIN, sl],
                                 start=True, stop=True)
                nc.scalar.copy(vt[:, sl], pv[:, :])

            # u = (W1-W2) @ featsl.T + bias  (plain bf16)
            for c in range(r // CH):
                sl = slice(c * CH, (c + 1) * CH)
                pu = psa.tile([COUT, CH], f32, tag="psa")
                nc.tensor.matmul(pu[:, :], wdt[:, :], oneH[0:CIN, sl],
                                 start=True, stop=True)
                nc.vector.tensor_scalar_add(ut[:, sl], pu[:, :], bias[:, :])

            # ---------------- main loop over row tiles ----------------
            # Software pipeline: stage A (PE keys matmuls + Act copies) for
            # tile rt+1 is emitted BEFORE stage B (DVE top-16 + gather chain)
            # of tile rt, so the per-engine program order has no cross-tile
            # stall: the tiny PE transpose of B(rt) — which waits on DVE —
            # sits after A(rt+1)'s matmuls, and the Pool gather stream stays
            # saturated.
            def stage_a(rt):
                rsl = slice(rt * 128, (rt + 1) * 128)
                keys = keysp.tile([128, n], f32, tag="keys")
                for c in range(nch):
                    sl = slice(c * CH, (c + 1) * CH)
                    pk = psk.tile([128, CH], f32, tag="psk")
                    nc.tensor.matmul(pk[:, :], oneH[:, rsl], augH[:, sl],
                                     start=True, stop=False)
                    nc.tensor.matmul(pk[:, :], oneH[:, rsl], augL[:, sl],
                                     start=False, stop=False)
                    nc.tensor.matmul(pk[:, :], oneL[:, rsl], augH[:, sl],
                                     start=False, stop=True)
                    nc.scalar.copy(keys[:, sl], pk[:, :])
                return keys

            def stage_b1(rt, keys):
                rsl = slice(rt * 128, (rt + 1) * 128)
                if minimal:
                    ot0 = med.tile([128, 128], f32, tag="ot")
                    nc.vector.tensor_add(ot0[:, :], keys[:, 0:128], ut[:, rsl])
                    nc.sync.dma_start(out_d.ap()[:, rsl], ot0[:, :])
                    return

                # jfw holds the 16 per-row indices replicated 8x along the
                # free dim: jfw[m, 16g+q] = j[m, q]. One PE transpose then
                # yields the wrapped ap_gather index layout directly
                # (widx[16g+q, m] = j[m, q]) with no partition-shifting DMAs.
                jfw = small.tile([128, 128], f32, tag="jfw")
                if no_topk:
                    nc.vector.memset(jfw[:, :], 5.0)
                else:
                    # stage A: top-8 of each of the 8 groups of 1024
                    gmax = small.tile([128, 8 * NG], f32, tag="gmax")
                    for g in range(NG):
                        nc.vector.max(gmax[:, 8 * g:8 * (g + 1)],
                                      keys[:, GS * g:GS * (g + 1)])
                    # stage B (64-wide): r1 = global top-8 values; suppress
                    # them (>= t8 -> -3e38); r2 = ranks 9-16
                    r1 = small.tile([128, 8], f32, tag="r8")
                    nc.vector.max(r1[:, :], gmax[:, :])
                    sup = small.tile([128, 8 * NG], f32, tag="sup")
                    nc.vector.tensor_scalar(sup[:, :], gmax[:, :],
                                            r1[:, 7:8], None, Alu.is_ge)
                    nc.vector.scalar_tensor_tensor(sup[:, :], sup[:, :],
                                                   -3.0e38, gmax[:, :],
                                                   Alu.mult, Alu.add)
                    r2 = small.tile([128, 8], f32, tag="r8")
                    nc.vector.max(r2[:, :], sup[:, :])
                    # index recovery: two full-width scans
                    i1 = small.tile([128, 8], u32, tag="i8")
                    nc.vector.max_index(i1[:, :], r1[:, :], keys[:, :])
                    i2 = small.tile([128, 8], u32, tag="i8")
                    nc.vector.max_index(i2[:, :], r2[:, :], keys[:, :])
                    for g in range(8):
                        nc.scalar.copy(jfw[:, 16 * g:16 * g + 8], i1[:, :])
                        nc.scalar.copy(jfw[:, 16 * g + 8:16 * g + 16], i2[:, :])

                tp = psa.tile([128, 128], f32, tag="tp")
                nc.tensor.transpose(tp[:, :], jfw[:, :], ident[:, :])
                widx = small.tile([128, 128], i16, tag="widx")
                nc.scalar.copy(widx[:, :], tp[:, :])

                if no_gather:
                    return None
                # gather v columns of the 2048 neighbors
                vg = vgp.tile([128, 2048], f32, tag="vg")
                nc.gpsimd.ap_gather(vg[:, :], vt[:, :], widx[:, :],
                                    channels=128, num_elems=n, d=1,
                                    num_idxs=2048)
                return vg

            def stage_b2(rt, vg, dep=None):
                rsl = slice(rt * 128, (rt + 1) * 128)
                mx = med.tile([128, 128], f32, tag="mx")
                if vg is None:
                    nc.vector.tensor_copy(mx[:, :], ut[:, rsl])
                else:
                    v3 = vg[:, :].rearrange("p (g k) -> p g k", k=K)
                    t1 = treep.tile([128, 1024], f32, tag="t1")
                    t1v = t1[:, :].rearrange("p (g k) -> p g k", k=8)
                    if dep is not None:
                        # bypass-read one scalar of the sibling tile's vg so
                        # this tree waits for BOTH gathers: the Q7 drain that
                        # precedes it (~11us on HW) is paid once per pair.
                        nc.vector.scalar_tensor_tensor(t1v, v3[:, :, 0:8],
                                                       dep[:, 0:1],
                                                       v3[:, :, 8:16],
                                                       Alu.bypass, Alu.max)
                    else:
                        nc.vector.tensor_tensor(t1v, v3[:, :, 0:8],
                                                v3[:, :, 8:16], Alu.max)
                    t2 = treep.tile([128, 512], f32, tag="t2")
                    t2v = t2[:, :].rearrange("p (g k) -> p g k", k=4)
                    nc.vector.tensor_tensor(t2v, t1v[:, :, 0:4],
                                            t1v[:, :, 4:8], Alu.max)
                    t3 = treep.tile([128, 256], f32, tag="t3")
                    t3v = t3[:, :].rearrange("p (g k) -> p g k", k=2)
                    nc.vector.tensor_tensor(t3v, t2v[:, :, 0:2],
                                            t2v[:, :, 2:4], Alu.max)
                    nc.vector.tensor_tensor(mx[:, :], t3v[:, :, 0],
                                            t3v[:, :, 1], Alu.max)
                ot = med.tile([128, 128], f32, tag="ot")
                nc.vector.tensor_add(ot[:, :], mx[:, :], ut[:, rsl])
                nc.sync.dma_start(out_d.ap()[:, rsl], ot[:, :])

            def main_body():
                keys0 = stage_a(0)
                keys1 = stage_a(1)
                for s in range(0, rt_count, 2):
                    vga = stage_b1(s, keys0)
                    vgb = stage_b1(s + 1, keys1)
                    if s + 2 < rt_count:
                        keys0 = stage_a(s + 2)
                        keys1 = stage_a(s + 3)
                    if minimal:
                        continue
                    stage_b2(s, vga, dep=vgb)
                    stage_b2(s + 1, vgb)

            if repeat > 1:
                with tc.For_i(0, repeat, 1):
                    main_body()
            else:
                main_body()

    nc.compile()
    return nc


def _get_program(n, r, num_devices):
    key = (n, r, num_devices)
    if key not in _prog_cache:
        _prog_cache[key] = build_program(n, r, num_devices)
    return _prog_cache[key]


def run_cores(feats_by_core, featsl_by_core, W, b, n, r, trace=False):
    """Run the SPMD program. feats_by_core[i]: [64, n]; featsl_by_core[i]: [64, r]."""
    from concourse.bass_utils import run_bass_kernel_spmd

    num = len(feats_by_core)
    W1 = W[:, :CIN]
    W2 = W[:, CIN:]
    w2t = np.ascontiguousarray(W2.T).astype(np.float32)
    wdt = np.ascontiguousarray((W1 - W2).T).astype(np.float32)
    bias = b.reshape(COUT, 1).astype(np.float32)
    ident = np.eye(128, dtype=np.float32)
    in_maps = []
    for i in range(num):
        in_maps.append({
            "feats": np.ascontiguousarray(feats_by_core[i], dtype=np.float32),
            "featsl": np.ascontiguousarray(featsl_by_core[i], dtype=np.float32),
            "w2t": w2t, "wdt": wdt, "bias": bias, "ident": ident,
        })
    nc = _get_program(n, r, num)
    res = run_bass_kernel_spmd(nc, in_maps, core_ids=list(range(num)), trace=trace)
    return [res.results[i]["out"] for i in range(num)], res


def kernel(x, W, b):
    """Full-input entry point: x [4, 64, 8192, 1] f32 -> [4, 128, 8192, 1] f32."""
    x = np.asarray(x, dtype=np.float32)
    W = np.asarray(W, dtype=np.float32)
    b = np.asarray(b, dtype=np.float32)
    xb = np.ascontiguousarray(x[:, :, :, 0])            # [4, 64, 8192]
    r = N // 2
    feats_by_core = []
    featsl_by_core = []
    for core in range(N_CORES):
        bi, half = core // 2, core % 2
        feats_by_core.append(xb[bi])
        featsl_by_core.append(xb[bi][:, half * r:(half + 1) * r])
    outs, _ = run_cores(feats_by_core, featsl_by_core, W, b, N, r)
    out = np.empty((B, COUT, N, 1), np.float32)
    for core in range(N_CORES):
        bi, half = core // 2, core % 2
        out[bi, :, half * r:(half + 1) * r, 0] = outs[core]
    return out


# revision 12
# speedup vs baseline: 1.1828x; 1.0927x over previous
"""Trainium2 Bass kernel for DynConv2d (DGCNN-style edge conv).

Reference computation (per batch b of 4):
  feats  = x[b,:,:,0].T                      # [N=8192, C=64]
  nn_idx = top16_j( 2*<f_i,f_j> - |f_i|^2 - |f_j|^2 )    # kNN graph
  edge   = [x_i, x_j - x_i] @ W.T + bias     # 1x1 conv, W [128, 128]
  out    = max over 16 neighbors             # -> [128, N]

Key algebraic reduction: with W = [W1 | W2],
  out[n, c] = u_n[c] + max_{j in top16(n)} v_j[c]
  u = (W1 - W2) @ feats.T + bias             # [128, N]
  v = W2 @ feats.T                           # [128, N]
so only a per-row top-16 over key[i, j] = <f_i, f_j> - 0.5*|f_j|^2 and a
gather+max over v remain.

Measured-HW design notes (per 128-row tile, per core):
 * Key matmuls: 3 accumulated bf16 matmuls on split operands
   (xh*yh + xh*yl + xl*yh; x = xh + xl + O(2^-17 x)) — bf16 matmul is
   ~0.5us vs 6.5us for fp32 on this part. Key error ~2^-17, far below
   typical top-16 decision gaps.
 * Top-16 selection (DVE): hierarchical. 8 group-wise max8 over 1024-wide
   groups (13.8us), then a 64-wide stage B: max8 -> top8 values, suppress
   them with is_ge mask + mult/add (match_replace has a ~5us fixed cost,
   the 2-op suppress is ~1us), max8 -> ranks 9-16. Index recovery with two
   full-width max_index scans (10.5us each). Replaces the flat 5-pass
   top-16 (50.5us) with ~38us.
 * v and u matmuls run in plain bf16 (error 2^-8 only lands on output
   values, tolerance is 2e-2).
 * Gather stays on gpsimd ap_gather (55us, runs parallel to DVE).

Sharding: 8 cores = 4 batches x 2 halves of N. Each core gets the full
feature matrix of its batch plus its local half of rows and produces
out[128, 4096]; the host concatenates. No collectives.
"""

import sys

for _p in ("/opt/trn_rl_repo", "/root/.axon_site/_ro/trn_rl_repo"):
    if _p not in sys.path:
        sys.path.insert(0, _p)

import numpy as np

B = 4
CIN = 64
COUT = 128
N = 8192
K = 16
N_CORES = 8

_prog_cache = {}


def build_program(n=N, r=N // 2, num_devices=N_CORES, repeat=1,
                  no_topk=False, no_gather=False, minimal=False,
                  tree_reduce=True):
    """Build + compile the SPMD bass program (same NEFF on all cores).

    repeat>1 wraps the main loop in a device-side For_i for benchmarking.
    no_topk/no_gather/minimal are benchmarking ablations (wrong results).
    tree_reduce: use 4 strided tensor_tensor max levels instead of
    reduce_max for the 16-neighbor max.
    """
    import concourse.bacc as bacc
    import concourse.mybir as mybir
    import concourse.tile as tile

    f32 = mybir.dt.float32
    bf16 = mybir.dt.bfloat16
    i16 = mybir.dt.int16
    u32 = mybir.dt.uint32
    Alu = mybir.AluOpType
    CH = 512
    nch = n // CH
    rt_count = r // 128
    NG = 8                    # groups for hierarchical top-16
    GS = n // NG              # group size (1024)

    nc = bacc.Bacc("TRN2", target_bir_lowering=False, debug=False,
                   num_devices=num_devices)

    feats_d = nc.dram_tensor("feats", [CIN, n], f32, kind="ExternalInput")
    featsl_d = nc.dram_tensor("featsl", [CIN, r], f32, kind="ExternalInput")
    w2t_d = nc.dram_tensor("w2t", [CIN, COUT], f32, kind="ExternalInput")
    wdt_d = nc.dram_tensor("wdt", [CIN, COUT], f32, kind="ExternalInput")
    bias_d = nc.dram_tensor("bias", [COUT, 1], f32, kind="ExternalInput")
    ident_d = nc.dram_tensor("ident", [128, 128], f32, kind="ExternalInput")
    out_d = nc.dram_tensor("out", [COUT, r], f32, kind="ExternalOutput")

    with tile.TileContext(nc) as tc:
        with tc.tile_pool(name="const", bufs=1) as const, \
             tc.tile_pool(name="keys", bufs=2) as keysp, \
             tc.tile_pool(name="vg", bufs=3) as vgp, \
             tc.tile_pool(name="small", bufs=3) as small, \
             tc.tile_pool(name="med", bufs=2) as med, \
             tc.tile_pool(name="tree", bufs=2) as treep, \
             tc.tile_pool(name="psk", bufs=4, space="PSUM") as psk, \
             tc.tile_pool(name="psa", bufs=2, space="PSUM") as psa:

            # ---------------- prologue ----------------
            # fp32 staging lives in borrowed "keys" slots (prologue only);
            # persistent tensors in the const pool.
            feats_aug = keysp.tile([CIN + 1, n], f32, tag="keys")
            nc.sync.dma_start(feats_aug[0:CIN, :], feats_d.ap())

            w2t = const.tile([CIN, COUT], bf16)
            w2f = med.tile([CIN, COUT], f32, tag="wstage")
            nc.sync.dma_start(w2f[:, :], w2t_d.ap())
            nc.vector.tensor_copy(w2t[:, :], w2f[:, :])
            wdt = const.tile([CIN, COUT], bf16)
            wdf = med.tile([CIN, COUT], f32, tag="wstage")
            nc.sync.dma_start(wdf[:, :], wdt_d.ap())
            nc.vector.tensor_copy(wdt[:, :], wdf[:, :])
            bias = const.tile([COUT, 1], f32)
            nc.sync.dma_start(bias[:, :], bias_d.ap())
            ident = const.tile([128, 128], f32)
            nc.sync.dma_start(ident[:, :], ident_d.ap())
            ones64 = const.tile([CIN, 1], f32)
            nc.vector.memset(ones64[:, :], 1.0)

            vt = const.tile([COUT, n], f32)
            ut = const.tile([COUT, r], bf16)

            # |f_j|^2 row: square, then fp32 ones-matmul partition sum
            featsq = keysp.tile([CIN + 1, n], f32, tag="keys")
            nc.scalar.activation(featsq[0:CIN, :], feats_aug[0:CIN, :],
                                 mybir.ActivationFunctionType.Square)
            for c in range(nch):
                sl = slice(c * CH, (c + 1) * CH)
                pxx = psa.tile([1, CH], f32, tag="psa")
                nc.tensor.matmul(pxx[:, :], ones64[:, :], featsq[0:CIN, sl],
                                 start=True, stop=True)
                xs = med.tile([1, CH], f32, tag="xs")
                nc.scalar.activation(xs[:, :], pxx[:, :],
                                     mybir.ActivationFunctionType.Copy, scale=-0.5)
                # DMA shifts partition base: row 64 of feats_aug = -0.5*xx
                nc.sync.dma_start(feats_aug[CIN:CIN + 1, sl], xs[:, :])

            # bf16 split operands for the key matmul: x = H + L + O(2^-17 x)
            augH = const.tile([CIN + 1, n], bf16)
            augL = const.tile([CIN + 1, n], bf16)
            nc.vector.tensor_copy(augH[:, :], feats_aug[:, :])
            nc.vector.tensor_sub(augL[:, :], feats_aug[:, :], augH[:, :])
            oneH = const.tile([CIN + 1, r], bf16)
            oneL = const.tile([CIN + 1, r], bf16)
            # local rows fp32 staging reuses featsq's slot
            featsl_f32 = featsq
            nc.sync.dma_start(featsl_f32[0:CIN, 0:r], featsl_d.ap())
            nc.scalar.copy(oneH[0:CIN, :], featsl_f32[0:CIN, 0:r])
            nc.vector.memset(oneH[CIN:CIN + 1, :], 1.0)
            nc.vector.memset(oneL[CIN:CIN + 1, :], 0.0)
            nc.vector.tensor_sub(oneL[0:CIN, :], featsl_f32[0:CIN, 0:r],
                                 oneH[0:CIN, :])

            # v = W2 @ feats.T  (plain bf16; feeds only output values)
            for c in range(nch):
                sl = slice(c * CH, (c + 1) * CH)
                pv = psa.tile([COUT, CH], f32, tag="psa")
                nc.tensor.matmul(pv[:, :], w2t[:, :], augH[0:CIN, sl],
                                 start=True, stop=True)
                nc.scalar.copy(vt[:, sl], pv[:, :])

            # u = (W1-W2) @ featsl.T + bias  (plain bf16)
            for c in range(r // CH):
                sl = slice(c * CH, (c + 1) * CH)
                pu = psa.tile([COUT, CH], f32, tag="psa")
                nc.tensor.matmul(pu[:, :], wdt[:, :], oneH[0:CIN, sl],
                                 start=True, stop=True)
                nc.vector.tensor_scalar_add(ut[:, sl], pu[:, :], bias[:, :])

            # ---------------- main loop over row tiles ----------------
            # Software pipeline: stage A (PE keys matmuls + Act copies) for
            # tile rt+1 is emitted BEFORE stage B (DVE top-16 + gather chain)
            # of tile rt, so the per-engine program order has no cross-tile
            # stall: the tiny PE transpose of B(rt) — which waits on DVE —
            # sits after A(rt+1)'s matmuls, and the Pool gather stream stays
            # saturated.
            def stage_a(rt):
                rsl = slice(rt * 128, (rt + 1) * 128)
                keys = keysp.tile([128, n], f32, tag="keys")
                for c in range(nch):
                    sl = slice(c * CH, (c + 1) * CH)
                    pk = psk.tile([128, CH], f32, tag="psk")
                    nc.tensor.matmul(pk[:, :], oneH[:, rsl], augH[:, sl],
                                     start=True, stop=False)
                    nc.tensor.matmul(pk[:, :], oneH[:, rsl], augL[:, sl],
                                     start=False, stop=False)
                    nc.tensor.matmul(pk[:, :], oneL[:, rsl], augH[:, sl],
                                     start=False, stop=True)
                    nc.scalar.copy(keys[:, sl], pk[:, :])
                return keys

            def stage_b1(rt, keys):
                rsl = slice(rt * 128, (rt + 1) * 128)
                if minimal:
                    ot0 = med.tile([128, 128], f32, tag="ot")
                    nc.vector.tensor_add(ot0[:, :], keys[:, 0:128], ut[:, rsl])
                    nc.sync.dma_start(out_d.ap()[:, rsl], ot0[:, :])
                    return

                # jfw holds the 16 per-row indices replicated 8x along the
                # free dim: jfw[m, 16g+q] = j[m, q]. One PE transpose then
                # yields the wrapped ap_gather index layout directly
                # (widx[16g+q, m] = j[m, q]) with no partition-shifting DMAs.
                jfw = small.tile([128, 128], f32, tag="jfw")
                if no_topk:
                    nc.vector.memset(jfw[:, :], 5.0)
                else:
                    # stage A: top-8 of each of the 8 groups of 1024
                    gmax = small.tile([128, 8 * NG], f32, tag="gmax")
                    for g in range(NG):
                        nc.vector.max(gmax[:, 8 * g:8 * (g + 1)],
                                      keys[:, GS * g:GS * (g + 1)])
                    # stage B (64-wide): r1 = global top-8 values; suppress
                    # them (>= t8 -> -3e38); r2 = ranks 9-16
                    r1 = small.tile([128, 8], f32, tag="r8")
                    nc.vector.max(r1[:, :], gmax[:, :])
                    sup = small.tile([128, 8 * NG], f32, tag="sup")
                    nc.vector.tensor_scalar(sup[:, :], gmax[:, :],
                                            r1[:, 7:8], None, Alu.is_ge)
                    nc.vector.scalar_tensor_tensor(sup[:, :], sup[:, :],
                                                   -3.0e38, gmax[:, :],
                                                   Alu.mult, Alu.add)
                    r2 = small.tile([128, 8], f32, tag="r8")
                    nc.vector.max(r2[:, :], sup[:, :])
                    # index recovery: two full-width scans
                    i1 = small.tile([128, 8], u32, tag="i8")
                    nc.vector.max_index(i1[:, :], r1[:, :], keys[:, :])
                    i2 = small.tile([128, 8], u32, tag="i8")
                    nc.vector.max_index(i2[:, :], r2[:, :], keys[:, :])
                    for g in range(8):
                        nc.scalar.copy(jfw[:, 16 * g:16 * g + 8], i1[:, :])
                        nc.scalar.copy(jfw[:, 16 * g + 8:16 * g + 16], i2[:, :])

                tp = psa.tile([128, 128], f32, tag="tp")
                nc.tensor.transpose(tp[:, :], jfw[:, :], ident[:, :])
                widx = small.tile([128, 128], i16, tag="widx")
                nc.scalar.copy(widx[:, :], tp[:, :])

                if no_gather:
                    return None
                # gather v columns of the 2048 neighbors
                vg = vgp.tile([128, 2048], f32, tag="vg")
                nc.gpsimd.ap_gather(vg[:, :], vt[:, :], widx[:, :],
                                    channels=128, num_elems=n, d=1,
                                    num_idxs=2048)
                return vg

            def stage_b2(rt, vg, dep=None):
                rsl = slice(rt * 128, (rt + 1) * 128)
                mx = med.tile([128, 128], f32, tag="mx")
                if vg is None:
                    nc.vector.tensor_copy(mx[:, :], ut[:, rsl])
                else:
                    v3 = vg[:, :].rearrange("p (g k) -> p g k", k=K)
                    t1 = treep.tile([128, 1024], f32, tag="t1")
                    t1v = t1[:, :].rearrange("p (g k) -> p g k", k=8)
                    if dep is not None:
                        # bypass-read one scalar of the sibling tile's vg so
                        # this tree waits for BOTH gathers: the Q7 drain that
                        # precedes it (~11us on HW) is paid once per pair.
                        nc.vector.scalar_tensor_tensor(t1v, v3[:, :, 0:8],
                                                       dep[:, 0:1],
                                                       v3[:, :, 8:16],
                                                       Alu.bypass, Alu.max)
                    else:
                        nc.vector.tensor_tensor(t1v, v3[:, :, 0:8],
                                                v3[:, :, 8:16], Alu.max)
                    t2 = treep.tile([128, 512], f32, tag="t2")
                    t2v = t2[:, :].rearrange("p (g k) -> p g k", k=4)
                    nc.vector.tensor_tensor(t2v, t1v[:, :, 0:4],
                                            t1v[:, :, 4:8], Alu.max)
                    t3 = treep.tile([128, 256], f32, tag="t3")
                    t3v = t3[:, :].rearrange("p (g k) -> p g k", k=2)
                    nc.vector.tensor_tensor(t3v, t2v[:, :, 0:2],
                                            t2v[:, :, 2:4], Alu.max)
                    nc.vector.tensor_tensor(mx[:, :], t3v[:, :, 0],
                                            t3v[:, :, 1], Alu.max)
                ot = med.tile([128, 128], f32, tag="ot")
                nc.vector.tensor_add(ot[:, :], mx[:, :], ut[:, rsl])
                nc.sync.dma_start(out_d.ap()[:, rsl], ot[:, :])

            def main_body():
                keys0 = stage_a(0)
                keys1 = stage_a(1)
                for s in range(0, rt_count, 2):
                    vga = stage_b1(s, keys0)
                    vgb = stage_b1(s + 1, keys1)
                    if s + 2 < rt_count:
                        keys0 = stage_a(s + 2)
                        keys1 = stage_a(s + 3)
                    if minimal:
                        continue
                    stage_b2(s, vga, dep=vgb)
                    stage_b2(s + 1, vgb)

            if repeat > 1:
                with tc.For_i(0, repeat, 1):
                    main_body()
            else:
                main_body()

    nc.compile()
    return nc


def _get_program(n, r, num_devices):
    key = (n, r, num_devices)
    if key not in _prog_cache:
        _prog_cache[key] = build_program(n, r, num_devices)
    return _prog_cache[key]


def run_cores(feats_by_core, featsl_by_core, W, b, n, r, trace=False):
    """Run the SPMD program. feats_by_core[i]: [64, n]; featsl_by_core[i]: [64, r]."""
    from concourse.bass_utils import run_bass_kernel_spmd

    num = len(feats_by_core)
    W1 = W[:, :CIN]
    W2 = W[:, CIN:]
    w2t = np.ascontiguousarray(W2.T).astype(np.float32)
    wdt = np.ascontiguousarray((W1 - W2).T).astype(np.float32)
    bias = b.reshape(COUT, 1).astype(np.float32)
    ident = np.eye(128, dtype=np.float32)
    in_maps = []
    for i in range(num):
        in_maps.append({
            "feats": np.ascontiguousarray(feats_by_core[i], dtype=np.float32),
            "featsl": np.ascontiguousarray(featsl_by_core[i], dtype=np.float32),
            "w2t": w2t, "wdt": wdt, "bias": bias, "ident": ident,
        })
    nc = _get_program(n, r, num)
    res = run_bass_kernel_spmd(nc, in_maps, core_ids=list(range(num)), trace=trace)
    return [res.results[i]["out"] for i in range(num)], res


def kernel(x, W, b):
    """Full-input entry point: x [4, 64, 8192, 1] f32 -> [4, 128, 8192, 1] f32."""
    x = np.asarray(x, dtype=np.float32)
    W = np.asarray(W, dtype=np.float32)
    b = np.asarray(b, dtype=np.float32)
    xb = np.ascontiguousarray(x[:, :, :, 0])            # [4, 64, 8192]
    r = N // 2
    feats_by_core = []
    featsl_by_core = []
    for core in range(N_CORES):
        bi, half = core // 2, core % 2
        feats_by_core.append(xb[bi])
        featsl_by_core.append(xb[bi][:, half * r:(half + 1) * r])
    outs, _ = run_cores(feats_by_core, featsl_by_core, W, b, N, r)
    out = np.empty((B, COUT, N, 1), np.float32)
    for core in range(N_CORES):
        bi, half = core // 2, core % 2
        out[bi, :, half * r:(half + 1) * r, 0] = outs[core]
    return out
